# revision 33
# baseline (speedup 1.0000x reference)
"""Mixture-of-Softmaxes kernel for 8 Trainium2 NeuronCores.

Strategy: tensor-parallel over the vocab dimension (V=100000 -> 12500/core).
Each core computes all B rows for its vocab shard: per-head logits via bf16
matmuls, exp via ScalarE (with row-sum side-accumulation), ONE [128,4]
per-block AllReduce of the 4 heads' softmax denominators (8 collectives
total instead of 32 -- the CC core serializes collectives at ~15us each, so
the per-head scheme's 32-deep collective chain dominated the old timeline),
then the pi-weighted mixture on VectorE. Output is gathered on the host by
concatenating the vocab shards (bf16 -> f32 cast on host).

Pipelining: exp values live in a 32-slot ring of [128,2048] tiles (24 per
block + 8 lookahead) so ScalarE/TensorE stream ~15us into the next block
while the current block's mixture waits on its collective. emb is streamed
from DRAM per chunk (freeing SBUF for the deep ring); projT and pi stay
resident. Queue placement avoids head-of-line blocks: emb stream on sync,
out-DMAs triggered from the vector queue right after their producing op,
stats DMAs + collectives on gpsimd.

Host-side prep: inputs are transposed (contraction dim -> SBUF partitions)
and cast to bf16; emb is additionally laid out chunk-major with the two
128-row k-groups side by side ([128, 2*cw] per chunk) so the kernel needs
no on-chip transposes.
"""

import numpy as np
import ml_dtypes

import concourse.bass as bass
import concourse.mybir as mybir
import concourse.tile as tile
from concourse import bacc
from concourse.bass_utils import run_bass_kernel_spmd
from concourse.bass_interp import get_hw_module

B, H, D, V = 1024, 4, 256, 100000
N_CORES = 8
V_S = V // N_CORES          # 12500 vocab entries per core
KT = D // 128               # 2 contraction k-tiles
BBLK = 128                  # b rows per block (= SBUF partitions)
N_BBLK = B // BBLK          # 8 blocks
CW = 2048                   # chunk width (one PSUM tile / ACT activate)
CHUNKS = [(i * CW, CW) for i in range(6)] + [(6 * CW, V_S - 6 * CW)]
N_CHUNK = len(CHUNKS)       # 7 (6 full + one 212-wide tail)
E_SLOTS = 37                # full e-ring: 24/block + 13 lookahead (~24us)
ET_SLOTS = 8                # tail e-ring: 4/block + 4 lookahead
EMB_SCALE = 16.0            # emb is cast to fp8e4 at 16x to keep precision

F32 = mybir.dt.float32
BF16 = mybir.dt.bfloat16
F8E4 = mybir.dt.float8e4

_RUN_KWARGS = {}  # test harness may set trace/tmpdir here
_CACHE = {}


def _build():
    nc = bacc.Bacc("TRN2", target_bir_lowering=False, debug=False,
                   num_devices=N_CORES)
    xT = nc.dram_tensor("xT", [D, B], BF16, kind="ExternalInput").ap()
    pmT = nc.dram_tensor("pmT", [D, H * D], BF16, kind="ExternalInput").ap()
    mmT = nc.dram_tensor("mmT", [D, H], BF16, kind="ExternalInput").ap()
    embT8 = nc.dram_tensor("embT8", [D, V_S], F8E4,
                           kind="ExternalInput").ap()
    out = nc.dram_tensor("out", [B, V_S], BF16, kind="ExternalOutput").ap()

    with tile.TileContext(nc) as tc:
        _body(tc, xT, pmT, mmT, embT8, out)
        tc._pool_ctx.close()

    nc.compile()
    nc.m = get_hw_module(nc.m)
    return nc


def _body(tc, xT, pmT, mmT, embT8, out):
    nc = tc.nc
    Exp = mybir.ActivationFunctionType.Exp
    Tanh = mybir.ActivationFunctionType.Tanh
    add = mybir.AluOpType.add

    import contextlib
    ctx = contextlib.ExitStack()
    tc._pool_ctx = ctx
    singles = ctx.enter_context(tc.tile_pool(name="singles", bufs=1))
    work = ctx.enter_context(tc.tile_pool(name="work", bufs=2))
    mix = ctx.enter_context(tc.tile_pool(name="mix", bufs=2))
    ering = ctx.enter_context(tc.tile_pool(name="ering", bufs=E_SLOTS))
    etail = ctx.enter_context(tc.tile_pool(name="etail", bufs=ET_SLOTS))
    psum = ctx.enter_context(tc.tile_pool(name="psum", bufs=2, space="PSUM"))
    dram = ctx.enter_context(tc.tile_pool(name="dram", bufs=4, space="DRAM"))

    # ---- SBUF inputs (xT/pmT borrow e-ring slots: prologue-only) ----
    sb_xT, sb_pmT, sb_mmT = [], [], []
    for k in range(KT):
        t = ering.tile([128, CW], BF16, tag="e", name=f"xT{k}")
        nc.sync.dma_start(out=t[:, :B], in_=xT[k * 128:(k + 1) * 128, :])
        sb_xT.append(t[:, :B])
        t = ering.tile([128, CW], BF16, tag="e", name=f"pmT{k}")
        nc.sync.dma_start(out=t[:, :H * D], in_=pmT[k * 128:(k + 1) * 128, :])
        sb_pmT.append(t[:, :H * D])
        t = singles.tile([128, H], BF16, tag=f"mmT{k}", name=f"mmT{k}")
        nc.sync.dma_start(out=t, in_=mmT[k * 128:(k + 1) * 128, :])
        sb_mmT.append(t)
    # ---- resident fp8 embedding shard (both k-groups, 25KB/partition).
    # Loaded AFTER xT/pmT/mmT on the sync queue: the prologue matmuls need
    # those first, and the 1.6MB emb DMAs would delay them ~10us ----
    sb_emb = []
    for k in range(KT):
        t = singles.tile([128, V_S], F8E4, tag=f"emb{k}", name=f"emb{k}")
        nc.sync.dma_start(out=t, in_=embT8[k * 128:(k + 1) * 128, :])
        sb_emb.append(t)

    # ---- startup barrier: absorbs one-time collective channel setup and
    # core start skew off the critical path (first real AllReduce otherwise
    # measures 25-60us vs ~12us steady-state) ----
    bar_in = dram.tile([128, 1], BF16, tag="barin", name="barin")
    bar_out = dram.tile([128, 1], BF16, tag="barout", name="barout")
    nc.gpsimd.dma_start(out=bar_in[:], in_=mmT[0:128, 0:1])
    nc.gpsimd.collective_compute(
        "AllReduce", add,
        replica_groups=[list(range(N_CORES))],
        ins=[bar_in.opt()], outs=[bar_out.opt()],
    )

    # ---- projT[h][kd] = tanh(proj_mat_h @ x.T), resident [128, B] bf16 ----
    projT = [[singles.tile([128, B], BF16, tag=f"pj{h}_{kd}",
                           name=f"pj{h}_{kd}")
              for kd in range(KT)] for h in range(H)]
    for h in range(H):
        for kd in range(KT):
            for bs in range(B // 512):
                ps = psum.tile([128, CW], F32, tag="ps", name="ps")
                for kc in range(KT):
                    nc.tensor.matmul(
                        ps[:, :512],
                        sb_pmT[kc][:, h * D + kd * 128: h * D + (kd + 1) * 128],
                        sb_xT[kc][:, bs * 512:(bs + 1) * 512],
                        start=(kc == 0), stop=(kc == KT - 1),
                    )
                nc.scalar.activation(
                    out=projT[h][kd][:, bs * 512:(bs + 1) * 512],
                    in_=ps[:, :512], func=Tanh)

    # ---- pi[b, h] = softmax_h(x @ mix_mat.T) per b-block ----
    sb_pi = []
    for i in range(N_BBLK):
        ps = psum.tile([128, CW], F32, tag="ps", name="ps")
        for kc in range(KT):
            nc.tensor.matmul(
                ps[:, :H],
                sb_xT[kc][:, i * 128:(i + 1) * 128],
                sb_mmT[kc],
                start=(kc == 0), stop=(kc == KT - 1),
            )
        m = work.tile([128, 1], F32, tag="pim", name="pim")
        nc.vector.tensor_reduce(out=m, in_=ps[:, :H],
                                axis=mybir.AxisListType.X,
                                op=mybir.AluOpType.max)
        negm = work.tile([128, 1], F32, tag="pinegm", name="pinegm")
        nc.vector.tensor_scalar_mul(negm, m, -1.0)
        e = work.tile([128, H], F32, tag="pie", name="pie")
        nc.scalar.activation(out=e, in_=ps[:, :H], func=Exp, bias=negm)
        s = work.tile([128, 1], F32, tag="pis", name="pis")
        nc.vector.tensor_reduce(out=s, in_=e, axis=mybir.AxisListType.X,
                                op=add)
        rs = work.tile([128, 1], F32, tag="pirs", name="pirs")
        nc.vector.reciprocal(rs, s)
        pi = singles.tile([128, H], F32, tag=f"pi{i}", name=f"pi{i}")
        nc.vector.tensor_scalar_mul(pi, e, rs)
        sb_pi.append(pi)

    # ---- helpers for the main loop ----
    def produce(i, h, ci, c0, cw, sparts, e_slots):
        if cw == CW:
            esl = ering.tile([128, CW], BF16, tag="e", name="e")
        else:
            esl = etail.tile([128, 256], BF16, tag="et", name="et")
        e_slots[(ci, h)] = esl
        ps = psum.tile([128, CW], F32, tag="ps", name="ps")
        for kc in range(KT):
            for n0 in range(0, cw, 512):
                nw = min(512, cw - n0)
                nc.tensor.matmul(
                    ps[:, n0:n0 + nw],
                    projT[h][kc][:, i * 128:(i + 1) * 128],
                    sb_emb[kc][:, c0 + n0:c0 + n0 + nw],
                    start=(kc == 0), stop=(kc == KT - 1),
                )
        nc.scalar.activation(
            out=esl[:, :cw], in_=ps[:, :cw], func=Exp,
            scale=1.0 / EMB_SCALE,
            accum_out=sparts[h][:, ci:ci + 1])

    def stats_collective(hs, sparts, w4, pi):
        """AllReduce the denominators for heads hs; writes w4[:, hs]."""
        n = len(hs)
        sl = work.tile([128, n], F32, tag=f"sl{hs[0]}", name=f"sl{hs[0]}")
        for j, h in enumerate(hs):
            # 7->1 reduce on ScalarE (Identity + accum_out): ready the
            # moment the last exp lands; on the DVE queue it would sit
            # behind the previous mixture (~10us head-of-line)
            scr = work.tile([128, N_CHUNK], F32, tag="scr", name="scr")
            nc.scalar.activation(
                out=scr, in_=sparts[h],
                func=mybir.ActivationFunctionType.Identity,
                accum_out=sl[:, j:j + 1])
        cc_in = dram.tile([128, n], F32, tag=f"ccin{hs[0]}", name="ccin")
        cc_out = dram.tile([128, n], F32, tag=f"ccout{hs[0]}", name="ccout")
        nc.gpsimd.dma_start(out=cc_in[:], in_=sl)
        nc.gpsimd.collective_compute(
            "AllReduce", add,
            replica_groups=[list(range(N_CORES))],
            ins=[cc_in.opt()], outs=[cc_out.opt()],
        )
        sg = work.tile([128, n], F32, tag=f"sg{hs[0]}", name=f"sg{hs[0]}")
        # gpsimd queue, NOT sync: avoids head-of-line blocking behind big
        # DMAs (latency-critical read)
        nc.gpsimd.dma_start(out=sg, in_=cc_out[:])
        rs = work.tile([128, n], F32, tag=f"rs{hs[0]}", name=f"rs{hs[0]}")
        nc.vector.reciprocal(rs, sg)
        nc.vector.tensor_mul(w4[:, hs[0]:hs[0] + n],
                             pi[:, hs[0]:hs[0] + n], rs)

    def mix_heads(i, hs, e_slots, w4, accs, acc_from_ring=False):
        """Accumulate heads hs into accs; DMA out after the last head."""
        for ci, (c0, cw) in enumerate(CHUNKS):
            for h in hs:
                if h == 0:
                    if acc_from_ring:
                        # last block: accs live across both collective
                        # rounds; borrow e-ring slots (same shape, and the
                        # ring has spare depth while the block winds down)
                        acc = ering.tile([128, CW], BF16, tag="e",
                                         name="acc")
                    else:
                        acc = mix.tile([128, CW], BF16, tag="acc",
                                       name="acc")
                    accs[ci] = acc
                    nc.vector.tensor_scalar_mul(
                        acc[:, :cw], e_slots[(ci, 0)][:, :cw], w4[:, 0:1])
                    continue
                eh = e_slots[(ci, h)]
                nc.vector.tensor_scalar_mul(eh[:, :cw], eh[:, :cw],
                                            w4[:, h:h + 1])
                nc.vector.tensor_tensor(
                    out=accs[ci][:, :cw], in0=accs[ci][:, :cw],
                    in1=eh[:, :cw], op=add)
            if hs[-1] == H - 1:
                # gpsimd queue: fires as the mixture produces each acc;
                # later stats ops enqueue after these complete anyway
                nc.gpsimd.dma_start(
                    out=out[i * 128:(i + 1) * 128, c0:c0 + cw],
                    in_=accs[ci][:, :cw])

    # ---- main loop over b-blocks ----
    for i in range(N_BBLK):
        sparts = [work.tile([128, N_CHUNK], F32, tag=f"sp{h}",
                            name=f"sp{h}") for h in range(H)]
        e_slots = {}
        w4 = work.tile([128, H], F32, tag="w4", name="w4")
        accs = {}
        if i < N_BBLK - 1:
            # chunk-major: each resident emb chunk feeds all 4 heads
            for ci, (c0, cw) in enumerate(CHUNKS):
                for h in range(H):
                    produce(i, h, ci, c0, cw, sparts, e_slots)
            stats_collective((0, 1, 2, 3), sparts, w4, sb_pi[i])
            mix_heads(i, (0, 1, 2, 3), e_slots, w4, accs)
        else:
            # LAST block: head-major with split collectives, so the h0/h1
            # AllReduce + mixture overlap h2/h3 production and only the
            # h2/h3 collective + half the mixture remain in the tail
            for h in (0, 1):
                for ci, (c0, cw) in enumerate(CHUNKS):
                    produce(i, h, ci, c0, cw, sparts, e_slots)
            stats_collective((0, 1), sparts, w4, sb_pi[i])
            mix_heads(i, (0, 1), e_slots, w4, accs, acc_from_ring=True)
            for h in (2, 3):
                for ci, (c0, cw) in enumerate(CHUNKS):
                    produce(i, h, ci, c0, cw, sparts, e_slots)
            stats_collective((2, 3), sparts, w4, sb_pi[i])
            mix_heads(i, (2, 3), e_slots, w4, accs)


def _get_nc():
    if "nc" not in _CACHE:
        _CACHE["nc"] = _build()
    return _CACHE["nc"]


def kernel(x, proj_mat, mix_mat, emb):
    nc = _get_nc()
    bf = ml_dtypes.bfloat16
    f8 = ml_dtypes.float8_e4m3
    xT = np.ascontiguousarray(x.astype(bf).T)
    pmT = np.ascontiguousarray(proj_mat.astype(bf).T)
    mmT = np.ascontiguousarray(mix_mat.astype(bf).T)
    emb8T = np.ascontiguousarray((emb * EMB_SCALE).astype(f8).T)
    in_maps = []
    for c in range(N_CORES):
        embT8 = np.ascontiguousarray(emb8T[:, c * V_S:(c + 1) * V_S])
        in_maps.append({"xT": xT, "pmT": pmT, "mmT": mmT, "embT8": embT8})
    res = run_bass_kernel_spmd(nc, in_maps, list(range(N_CORES)),
                               **_RUN_KWARGS)
    _CACHE["last_result"] = res
    return np.concatenate(
        [res.results[c]["out"].astype(np.float32) for c in range(N_CORES)],
        axis=1)


# revision 34
# speedup vs baseline: 1.0185x; 1.0185x over previous
"""Mixture-of-Softmaxes kernel for 8 Trainium2 NeuronCores.

Strategy: tensor-parallel over the vocab dimension (V=100000 -> 12500/core).
Each core computes all B rows for its vocab shard: per-head logits via
bf16(proj) x fp8e4(emb, scaled 16x) matmuls, exp on ScalarE (descale via
the free activation `scale`, row sums via accum_out), ONE [128,4] per-block
AllReduce of the 4 heads' softmax denominators (8 collectives instead of 32
-- the CC core serializes collectives at ~15us each, so a per-head scheme's
32-deep collective chain dominates the timeline), then the pi-weighted
mixture on VectorE. Output is gathered on the host by concatenating the
vocab shards (bf16 -> f32 cast on host).

Key engineering points (from trace analysis):
- emb stays RESIDENT in SBUF as fp8 (25KB/partition). Streaming it per
  block put ~630GB/s of aggregate read traffic on the chip's ~716GB/s HBM,
  which caused heavy DMA jitter, cross-core skew, and 2x collective
  variance.
- exp values live in a 37-slot ring of [128,2048] tiles (24 per block + 13
  lookahead) so ScalarE/TensorE stream ~24us into the next block while the
  current block's mixture waits on its collective.
- the per-head 7->1 denominator reduce runs on ScalarE (Identity +
  accum_out), not the DVE queue, where it would sit ~10us behind the
  previous block's mixture and delay every collective (cascading).
- a dummy AllReduce at kernel start absorbs the one-time collective
  channel-setup cost (~25-60us) off the critical path.
- the LAST block runs head-major with split (h0,h1)/(h2,h3) collectives:
  the first collective + half the mixture overlap the h2/h3 production,
  roughly halving the end-of-kernel serial tail.
- stats DMAs + collectives + out-DMAs ride the gpsimd queue; input loads
  ride sync; neither queue ever holds a long-latency-gated trigger in
  front of a latency-critical one.

Host-side prep: inputs are transposed (contraction dim -> SBUF partitions)
and cast to bf16 (emb: scaled 16x and cast to fp8e4) so the kernel needs
no on-chip transposes.
"""

import numpy as np
import ml_dtypes

import concourse.bass as bass
import concourse.mybir as mybir
import concourse.tile as tile
from concourse import bacc
from concourse.bass_utils import run_bass_kernel_spmd
from concourse.bass_interp import get_hw_module

B, H, D, V = 1024, 4, 256, 100000
N_CORES = 8
V_S = V // N_CORES          # 12500 vocab entries per core
KT = D // 128               # 2 contraction k-tiles
BBLK = 128                  # b rows per block (= SBUF partitions)
N_BBLK = B // BBLK          # 8 blocks
CW = 2048                   # chunk width (one PSUM tile / ACT activate)
CHUNKS = [(i * CW, CW) for i in range(6)] + [(6 * CW, V_S - 6 * CW)]
N_CHUNK = len(CHUNKS)       # 7 (6 full + one 212-wide tail)
E_SLOTS = 37                # full e-ring: 24/block + 13 lookahead (~24us)
ET_SLOTS = 8                # tail e-ring: 4/block + 4 lookahead
EMB_SCALE = 16.0            # emb is cast to fp8e4 at 16x to keep precision

F32 = mybir.dt.float32
BF16 = mybir.dt.bfloat16
F8E4 = mybir.dt.float8e4

_RUN_KWARGS = {}  # test harness may set trace/tmpdir here
_CACHE = {}


def _build():
    nc = bacc.Bacc("TRN2", target_bir_lowering=False, debug=False,
                   num_devices=N_CORES)
    xT = nc.dram_tensor("xT", [D, B], BF16, kind="ExternalInput").ap()
    pmT = nc.dram_tensor("pmT", [D, H * D], BF16, kind="ExternalInput").ap()
    mmT = nc.dram_tensor("mmT", [D, H], BF16, kind="ExternalInput").ap()
    embT8 = nc.dram_tensor("embT8", [D, V_S], F8E4,
                           kind="ExternalInput").ap()
    out = nc.dram_tensor("out", [B, V_S], BF16, kind="ExternalOutput").ap()

    with tile.TileContext(nc) as tc:
        _body(tc, xT, pmT, mmT, embT8, out)
        tc._pool_ctx.close()

    nc.compile()
    nc.m = get_hw_module(nc.m)
    return nc


def _body(tc, xT, pmT, mmT, embT8, out):
    nc = tc.nc
    Exp = mybir.ActivationFunctionType.Exp
    Tanh = mybir.ActivationFunctionType.Tanh
    add = mybir.AluOpType.add

    import contextlib
    ctx = contextlib.ExitStack()
    tc._pool_ctx = ctx
    singles = ctx.enter_context(tc.tile_pool(name="singles", bufs=1))
    work = ctx.enter_context(tc.tile_pool(name="work", bufs=2))
    mix = ctx.enter_context(tc.tile_pool(name="mix", bufs=2))
    ering = ctx.enter_context(tc.tile_pool(name="ering", bufs=E_SLOTS))
    etail = ctx.enter_context(tc.tile_pool(name="etail", bufs=ET_SLOTS))
    psum = ctx.enter_context(tc.tile_pool(name="psum", bufs=2, space="PSUM"))
    dram = ctx.enter_context(tc.tile_pool(name="dram", bufs=4, space="DRAM"))

    # ---- SBUF inputs (xT/pmT borrow e-ring slots: prologue-only) ----
    sb_xT, sb_pmT, sb_mmT = [], [], []
    for k in range(KT):
        t = ering.tile([128, CW], BF16, tag="e", name=f"xT{k}")
        nc.sync.dma_start(out=t[:, :B], in_=xT[k * 128:(k + 1) * 128, :])
        sb_xT.append(t[:, :B])
        t = ering.tile([128, CW], BF16, tag="e", name=f"pmT{k}")
        nc.sync.dma_start(out=t[:, :H * D], in_=pmT[k * 128:(k + 1) * 128, :])
        sb_pmT.append(t[:, :H * D])
        t = singles.tile([128, H], BF16, tag=f"mmT{k}", name=f"mmT{k}")
        nc.sync.dma_start(out=t, in_=mmT[k * 128:(k + 1) * 128, :])
        sb_mmT.append(t)
    # ---- resident fp8 embedding shard (both k-groups, 25KB/partition).
    # Loaded AFTER xT/pmT/mmT on the sync queue: the prologue matmuls need
    # those first, and the 1.6MB emb DMAs would delay them ~10us ----
    sb_emb = []
    for k in range(KT):
        t = singles.tile([128, V_S], F8E4, tag=f"emb{k}", name=f"emb{k}")
        nc.sync.dma_start(out=t, in_=embT8[k * 128:(k + 1) * 128, :])
        sb_emb.append(t)

    # ---- startup barrier: absorbs one-time collective channel setup and
    # core start skew off the critical path (first real AllReduce otherwise
    # measures 25-60us vs ~12us steady-state) ----
    bar_in = dram.tile([128, 1], BF16, tag="barin", name="barin")
    bar_out = dram.tile([128, 1], BF16, tag="barout", name="barout")
    nc.gpsimd.dma_start(out=bar_in[:], in_=mmT[0:128, 0:1])
    nc.gpsimd.collective_compute(
        "AllReduce", add,
        replica_groups=[list(range(N_CORES))],
        ins=[bar_in.opt()], outs=[bar_out.opt()],
    )

    # ---- projT[h][kd] = tanh(proj_mat_h @ x.T), resident [128, B] bf16 ----
    projT = [[singles.tile([128, B], BF16, tag=f"pj{h}_{kd}",
                           name=f"pj{h}_{kd}")
              for kd in range(KT)] for h in range(H)]
    for h in range(H):
        for kd in range(KT):
            for bs in range(B // 512):
                ps = psum.tile([128, CW], F32, tag="ps", name="ps")
                for kc in range(KT):
                    nc.tensor.matmul(
                        ps[:, :512],
                        sb_pmT[kc][:, h * D + kd * 128: h * D + (kd + 1) * 128],
                        sb_xT[kc][:, bs * 512:(bs + 1) * 512],
                        start=(kc == 0), stop=(kc == KT - 1),
                    )
                nc.scalar.activation(
                    out=projT[h][kd][:, bs * 512:(bs + 1) * 512],
                    in_=ps[:, :512], func=Tanh)

    # ---- pi[b, h] = softmax_h(x @ mix_mat.T) per b-block ----
    sb_pi = []
    for i in range(N_BBLK):
        ps = psum.tile([128, CW], F32, tag="ps", name="ps")
        for kc in range(KT):
            nc.tensor.matmul(
                ps[:, :H],
                sb_xT[kc][:, i * 128:(i + 1) * 128],
                sb_mmT[kc],
                start=(kc == 0), stop=(kc == KT - 1),
            )
        m = work.tile([128, 1], F32, tag="pim", name="pim")
        nc.vector.tensor_reduce(out=m, in_=ps[:, :H],
                                axis=mybir.AxisListType.X,
                                op=mybir.AluOpType.max)
        negm = work.tile([128, 1], F32, tag="pinegm", name="pinegm")
        nc.vector.tensor_scalar_mul(negm, m, -1.0)
        e = work.tile([128, H], F32, tag="pie", name="pie")
        nc.scalar.activation(out=e, in_=ps[:, :H], func=Exp, bias=negm)
        s = work.tile([128, 1], F32, tag="pis", name="pis")
        nc.vector.tensor_reduce(out=s, in_=e, axis=mybir.AxisListType.X,
                                op=add)
        rs = work.tile([128, 1], F32, tag="pirs", name="pirs")
        nc.vector.reciprocal(rs, s)
        pi = singles.tile([128, H], F32, tag=f"pi{i}", name=f"pi{i}")
        nc.vector.tensor_scalar_mul(pi, e, rs)
        sb_pi.append(pi)

    # ---- helpers for the main loop ----
    def produce(i, h, ci, c0, cw, sparts, e_slots):
        if cw == CW:
            esl = ering.tile([128, CW], BF16, tag="e", name="e")
        else:
            esl = etail.tile([128, 256], BF16, tag="et", name="et")
        e_slots[(ci, h)] = esl
        ps = psum.tile([128, CW], F32, tag="ps", name="ps")
        for kc in range(KT):
            for n0 in range(0, cw, 512):
                nw = min(512, cw - n0)
                nc.tensor.matmul(
                    ps[:, n0:n0 + nw],
                    projT[h][kc][:, i * 128:(i + 1) * 128],
                    sb_emb[kc][:, c0 + n0:c0 + n0 + nw],
                    start=(kc == 0), stop=(kc == KT - 1),
                )
        nc.scalar.activation(
            out=esl[:, :cw], in_=ps[:, :cw], func=Exp,
            scale=1.0 / EMB_SCALE,
            accum_out=sparts[h][:, ci:ci + 1])

    def stats_collective(hs, sparts, w4, pi):
        """AllReduce the denominators for heads hs; writes w4[:, hs]."""
        n = len(hs)
        sl = work.tile([128, n], F32, tag=f"sl{hs[0]}", name=f"sl{hs[0]}")
        for j, h in enumerate(hs):
            # 7->1 reduce on ScalarE (Identity + accum_out): ready the
            # moment the last exp lands; on the DVE queue it would sit
            # behind the previous mixture (~10us head-of-line)
            scr = work.tile([128, N_CHUNK], F32, tag="scr", name="scr")
            nc.scalar.activation(
                out=scr, in_=sparts[h],
                func=mybir.ActivationFunctionType.Identity,
                accum_out=sl[:, j:j + 1])
        cc_in = dram.tile([128, n], F32, tag=f"ccin{hs[0]}", name="ccin")
        cc_out = dram.tile([128, n], F32, tag=f"ccout{hs[0]}", name="ccout")
        nc.gpsimd.dma_start(out=cc_in[:], in_=sl)
        nc.gpsimd.collective_compute(
            "AllReduce", add,
            replica_groups=[list(range(N_CORES))],
            ins=[cc_in.opt()], outs=[cc_out.opt()],
        )
        sg = work.tile([128, n], F32, tag=f"sg{hs[0]}", name=f"sg{hs[0]}")
        # gpsimd queue, NOT sync: avoids head-of-line blocking behind big
        # DMAs (latency-critical read)
        nc.gpsimd.dma_start(out=sg, in_=cc_out[:])
        rs = work.tile([128, n], F32, tag=f"rs{hs[0]}", name=f"rs{hs[0]}")
        nc.vector.reciprocal(rs, sg)
        nc.vector.tensor_mul(w4[:, hs[0]:hs[0] + n],
                             pi[:, hs[0]:hs[0] + n], rs)

    def mix_heads(i, hs, e_slots, w4, accs, acc_from_ring=False):
        """Accumulate heads hs into accs; DMA out after the last head."""
        for ci, (c0, cw) in enumerate(CHUNKS):
            for h in hs:
                if h == 0:
                    if acc_from_ring:
                        # last block: accs live across both collective
                        # rounds; borrow e-ring slots (same shape, and the
                        # ring has spare depth while the block winds down)
                        acc = ering.tile([128, CW], BF16, tag="e",
                                         name="acc")
                    else:
                        acc = mix.tile([128, CW], BF16, tag="acc",
                                       name="acc")
                    accs[ci] = acc
                    nc.vector.tensor_scalar_mul(
                        acc[:, :cw], e_slots[(ci, 0)][:, :cw], w4[:, 0:1])
                    continue
                eh = e_slots[(ci, h)]
                nc.vector.tensor_scalar_mul(eh[:, :cw], eh[:, :cw],
                                            w4[:, h:h + 1])
                nc.vector.tensor_tensor(
                    out=accs[ci][:, :cw], in0=accs[ci][:, :cw],
                    in1=eh[:, :cw], op=add)
            if hs[-1] == H - 1:
                # gpsimd queue: fires as the mixture produces each acc;
                # later stats ops enqueue after these complete anyway
                nc.gpsimd.dma_start(
                    out=out[i * 128:(i + 1) * 128, c0:c0 + cw],
                    in_=accs[ci][:, :cw])

    # ---- main loop over b-blocks ----
    for i in range(N_BBLK):
        sparts = [work.tile([128, N_CHUNK], F32, tag=f"sp{h}",
                            name=f"sp{h}") for h in range(H)]
        e_slots = {}
        w4 = work.tile([128, H], F32, tag="w4", name="w4")
        accs = {}
        if i < N_BBLK - 1:
            # chunk-major: each resident emb chunk feeds all 4 heads
            for ci, (c0, cw) in enumerate(CHUNKS):
                for h in range(H):
                    produce(i, h, ci, c0, cw, sparts, e_slots)
            stats_collective((0, 1, 2, 3), sparts, w4, sb_pi[i])
            mix_heads(i, (0, 1, 2, 3), e_slots, w4, accs)
        else:
            # LAST block: head-major with split collectives, so the h0/h1
            # AllReduce + mixture overlap h2/h3 production and only the
            # h2/h3 collective + half the mixture remain in the tail
            for h in (0, 1):
                for ci, (c0, cw) in enumerate(CHUNKS):
                    produce(i, h, ci, c0, cw, sparts, e_slots)
            stats_collective((0, 1), sparts, w4, sb_pi[i])
            mix_heads(i, (0, 1), e_slots, w4, accs, acc_from_ring=True)
            for h in (2, 3):
                for ci, (c0, cw) in enumerate(CHUNKS):
                    produce(i, h, ci, c0, cw, sparts, e_slots)
            stats_collective((2, 3), sparts, w4, sb_pi[i])
            mix_heads(i, (2, 3), e_slots, w4, accs)


def _get_nc():
    if "nc" not in _CACHE:
        _CACHE["nc"] = _build()
    return _CACHE["nc"]


def kernel(x, proj_mat, mix_mat, emb):
    nc = _get_nc()
    bf = ml_dtypes.bfloat16
    f8 = ml_dtypes.float8_e4m3
    xT = np.ascontiguousarray(x.astype(bf).T)
    pmT = np.ascontiguousarray(proj_mat.astype(bf).T)
    mmT = np.ascontiguousarray(mix_mat.astype(bf).T)
    emb8T = np.ascontiguousarray((emb * EMB_SCALE).astype(f8).T)
    in_maps = []
    for c in range(N_CORES):
        embT8 = np.ascontiguousarray(emb8T[:, c * V_S:(c + 1) * V_S])
        in_maps.append({"xT": xT, "pmT": pmT, "mmT": mmT, "embT8": embT8})
    res = run_bass_kernel_spmd(nc, in_maps, list(range(N_CORES)),
                               **_RUN_KWARGS)
    _CACHE["last_result"] = res
    return np.concatenate(
        [res.results[c]["out"].astype(np.float32) for c in range(N_CORES)],
        axis=1)
